# revision 11
# baseline (speedup 1.0000x reference)
"""4-layer GCN (out = adj @ (h @ W) + b, stacked) on 8 trn2 NeuronCores.

Strategy (row-parallel over nodes, host-prepped adjacency):
  - Each core owns R = N/8 rows of adj (its output rows for every layer).
  - The adjacency is transposed and quantized ON THE HOST into two
    device-resident copies: fp8 e4m3 (scaled by 2^16 so values land in
    [0, 4]) for layers 0-2, and bf16 for the final layer.  Quantization
    noise from intermediate layers is damped ~100-1000x by the averaging
    structure of adj (top singular vector ~ ones), so only the final
    layer needs bf16; fp8 halves its HBM traffic.
  - Per layer the core streams its adjT shard in 2 MiB chunks and runs
    the big GEMM h^T = Z^T-contracted against adjT.  fp8 layers feed
    the PE MIXED operands: bf16 Z (stationary) x fp8 adjT (moving) --
    quantizing Z itself to e4m3 costs 2.5e-2 rel err (measured), while
    fp8 adjT alone costs under 2e-4, so Z stays bf16.
  - Z = h @ W is computed redundantly per core (tiny) in bf16; the fp8
    adjacency's 2^16 scale is removed by a mult fused into the
    PSUM->SBUF bias add (tensor_scalar mult+add).
  - h^T shards are AllGather'd (bf16) between layers.

kernel(**inputs) takes the full-size numpy inputs and returns the full
[N, 16] float32 output.
"""

import os

import numpy as np
import ml_dtypes

P = 128            # SBUF partitions / PE tile size
N_CORES = 8
SEG = 512          # fp32 PSUM bank width (free-dim elements)

# Full-problem config (must match the harness problem)
FULL_N = 16384
FULL_D_IN = 128
FULL_D_HID = 64
FULL_N_CLASSES = 16
FULL_N_HIDDEN_LAYERS = 2

ADJ_SHIFT = 16     # adjT8 = e4m3(adjT * 2^ADJ_SHIFT); adj max = 1/N = 2^-14
CHUNK8 = 8         # fp8 k-blocks per strip DMA  (128p x 8 x 2048 x 1B = 2 MiB)
CHUNK16 = 4        # bf16 k-blocks per strip DMA (128p x 4 x 2048 x 2B = 2 MiB)

_CACHE = {}
_LAST_RESULTS = None  # BassKernelResults of the most recent run (for test.py)


def _split_dma_waits(nc, mybir, max_waits=1, noop_waits=1):
    """Walrus' DMA pseudo-instruction supports at most 2 sem waits; Tile can
    emit 3+.  Hoist all waits of offending DMAs onto a NoOp on the issuing
    engine immediately before the DMA (same NX stream, so ordering holds)."""
    for f in nc.m.functions:
        for bb in f.blocks:
            insts = bb.instructions
            i = 0
            while i < len(insts):
                ins = insts[i]
                si = ins.sync_info
                if (
                    si is not None
                    and si.on_wait
                    and len(si.on_wait) > max_waits
                ):
                    waits = list(si.on_wait)
                    keep = waits[-max_waits:]
                    extra = waits[:-max_waits]
                    for j in range(0, len(extra), noop_waits):
                        noop = mybir.InstNoOp(
                            name=nc.get_next_instruction_name(),
                            engine=ins.engine,
                            ins=[],
                            outs=[],
                            sync_info=mybir.SyncInfo(
                                on_wait=extra[j : j + noop_waits], on_update=[]
                            ),
                        )
                        insts.insert(i, noop)
                        i += 1
                    ins.sync_info = mybir.SyncInfo(
                        on_wait=keep, on_update=list(si.on_update or [])
                    )
                i += 1


def _build(N, R, layer_dims, collectives=True, split_waits=True):
    """Build the per-core Bass program.

    N: total nodes; R: rows per core; layer_dims: [(d_in, d_out), ...]
    """
    import concourse.bass as bass
    import concourse.mybir as mybir
    from concourse import tile

    f32 = mybir.dt.float32
    bf16 = mybir.dt.bfloat16
    fp8 = mybir.dt.float8e4

    KB = N // P                    # contraction k-blocks
    n_seg = R // SEG
    n_layers = len(layer_dims)
    d_in0 = layer_dims[0][0]
    d_last = layer_dims[-1][1]

    nc = bass.Bass(trn_type="TRN2", num_devices=N_CORES)

    adjT8_d = nc.dram_tensor("adjT8", [N, R], fp8, kind="ExternalInput")
    adjT16_d = nc.dram_tensor("adjT16", [N, R], bf16, kind="ExternalInput")
    xT_d = nc.dram_tensor("xT", [d_in0, N], bf16, kind="ExternalInput")
    w_d = [
        nc.dram_tensor(f"w{l}", [di, do], bf16, kind="ExternalInput")
        for l, (di, do) in enumerate(layer_dims)
    ]
    b_d = [
        nc.dram_tensor(f"b{l}", [do, 1], f32, kind="ExternalInput")
        for l, (di, do) in enumerate(layer_dims)
    ]
    outT_d = nc.dram_tensor("outT", [d_last, R], f32, kind="ExternalOutput")

    with tile.TileContext(nc) as tc:
        with (
            tc.tile_pool(name="const", bufs=1) as constp,
            tc.tile_pool(name="xt", bufs=1) as xtp,
            tc.tile_pool(name="z16", bufs=1) as z16p,
            tc.tile_pool(name="s8", bufs=3) as s8p,
            tc.tile_pool(name="s16", bufs=2) as s16p,
            tc.tile_pool(name="h", bufs=2) as hp,
            tc.tile_pool(name="hfull", bufs=1) as hfp,
            tc.tile_pool(name="pz", bufs=2, space="PSUM") as pzp,
            tc.tile_pool(name="ph", bufs=1, space="PSUM") as php,
            tc.tile_pool(name="dram", bufs=1, space="DRAM") as dramp,
        ):
            w_sb, b_sb = [], []
            for l, (di, do) in enumerate(layer_dims):
                w = constp.tile([di, do], bf16, tag=f"w{l}")
                nc.sync.dma_start(w[:], w_d[l][:])
                b = constp.tile([do, 1], f32, tag=f"b{l}")
                nc.sync.dma_start(b[:], b_d[l][:])
                w_sb.append(w)
                b_sb.append(b)

            # x^T replicated; serves as h0^T for the layer-0 Z stage.
            xt = xtp.tile([d_in0, N], bf16, tag="xt")
            nc.sync.dma_start(xt[:], xT_d[:])

            hT_bf = None  # gathered h^T [d, N] bf16 for layers >= 1
            for l in range(n_layers):
                di, do = layer_dims[l]
                last = l == n_layers - 1
                use_fp8 = not last

                # ---- Z_l = h_l @ W_l, [k-part, kb, do] layout, bf16 ----
                hsrc = xt if l == 0 else hT_bf
                zbuf = z16p.tile([P, KB, do], bf16, tag="zbuf")
                for kb in range(KB):
                    pz = pzp.tile([P, do], f32, tag="pz")
                    nc.tensor.matmul(
                        pz[:],
                        hsrc[:, kb * P : (kb + 1) * P],
                        w_sb[l][:],
                        start=True,
                        stop=True,
                    )
                    nc.any.tensor_copy(zbuf[:, kb, :], pz[:])

                # ---- big GEMM: h_{l+1}^T[n, i] = sum_k Z[k, n] adjT[k, i] ----
                ph = php.tile([do, R], f32, tag="ph")
                chunk = CHUNK8 if use_fp8 else CHUNK16
                n_chunks = KB // chunk
                for c in range(n_chunks):
                    kb0 = c * chunk
                    if use_fp8:
                        strip = s8p.tile([P, chunk, R], fp8, tag="s8")
                        src = adjT8_d
                    else:
                        strip = s16p.tile([P, chunk, R], bf16, tag="s16")
                        src = adjT16_d
                    nc.sync.dma_start(
                        strip[:],
                        src[kb0 * P : (kb0 + chunk) * P, :].rearrange(
                            "(kk p) r -> p kk r", p=P
                        ),
                    )
                    for j in range(chunk):
                        kb = kb0 + j
                        for s in range(n_seg):
                            nc.tensor.matmul(
                                ph[:, s * SEG : (s + 1) * SEG],
                                zbuf[:, kb, :],
                                strip[:, j, s * SEG : (s + 1) * SEG],
                                start=(kb == 0),
                                stop=(kb == KB - 1),
                            )

                # ---- descale + bias add and inter-layer AllGather ----
                if last:
                    hf = hp.tile([do, R], f32, tag="hf")
                    for s in range(n_seg):
                        nc.vector.tensor_scalar_add(
                            hf[:, s * SEG : (s + 1) * SEG],
                            ph[:, s * SEG : (s + 1) * SEG],
                            b_sb[l][:, 0:1],
                        )
                    nc.sync.dma_start(outT_d[:], hf[:])
                else:
                    descale = 2.0 ** -ADJ_SHIFT
                    hb = hp.tile([do, R], bf16, tag="hb")
                    for s in range(n_seg):
                        nc.vector.tensor_scalar(
                            hb[:, s * SEG : (s + 1) * SEG],
                            ph[:, s * SEG : (s + 1) * SEG],
                            descale,
                            b_sb[l][:, 0:1],
                            op0=mybir.AluOpType.mult,
                            op1=mybir.AluOpType.add,
                        )
                    if collectives:
                        cc_in = dramp.tile([do, R], bf16, tag=f"ccin{l}")
                        nc.sync.dma_start(cc_in[:], hb[:])
                        cc_out = dramp.tile(
                            [N_CORES * do, R], bf16, addr_space="Shared",
                            tag=f"ccout{l}",
                        )
                        nc.gpsimd.collective_compute(
                            "AllGather",
                            mybir.AluOpType.bypass,
                            replica_groups=[list(range(N_CORES))],
                            ins=[cc_in.opt()],
                            outs=[cc_out.opt()],
                        )
                        hT_bf = hfp.tile([do, N], bf16, tag="hfull")
                        nc.sync.dma_start(
                            hT_bf[:].rearrange("d (r i) -> d r i", i=R),
                            cc_out[:].rearrange("(r d) i -> d r i", d=do),
                        )
                    else:
                        # single-core timing build: fake the gather with a
                        # DRAM round-trip of the same shape
                        cc_in = dramp.tile([do, R], bf16, tag=f"ccin{l}")
                        nc.sync.dma_start(cc_in[:], hb[:])
                        hT_bf = hfp.tile([do, N], bf16, tag="hfull")
                        for r in range(N_CORES):
                            nc.sync.dma_start(
                                hT_bf[:, r * R : (r + 1) * R], cc_in[:]
                            )
    if split_waits:
        _split_dma_waits(nc, mybir)
    return nc


def _prep_inputs(x, adj, W_in, b_in, W_hidden, b_hidden, W_out, b_out, N, R):
    bf = ml_dtypes.bfloat16
    f8 = ml_dtypes.float8_e4m3
    xT = np.ascontiguousarray(np.asarray(x, dtype=np.float32).T).astype(bf)
    ws = (
        [np.asarray(W_in)]
        + [np.asarray(W_hidden)[i] for i in range(np.asarray(W_hidden).shape[0])]
        + [np.asarray(W_out)]
    )
    bs = (
        [np.asarray(b_in)]
        + [np.asarray(b_hidden)[i] for i in range(np.asarray(b_hidden).shape[0])]
        + [np.asarray(b_out)]
    )
    ws = [np.ascontiguousarray(w.astype(np.float32)).astype(bf) for w in ws]
    bs = [np.ascontiguousarray(b.astype(np.float32).reshape(-1, 1)) for b in bs]

    adjT = np.asarray(adj, dtype=np.float32).T
    adjT8 = (adjT * float(2.0 ** ADJ_SHIFT)).astype(f8)
    adjT16 = adjT.astype(bf)
    in_maps = []
    for c in range(N_CORES):
        m = {
            "adjT8": np.ascontiguousarray(adjT8[:, c * R : (c + 1) * R]),
            "adjT16": np.ascontiguousarray(adjT16[:, c * R : (c + 1) * R]),
            "xT": xT,
        }
        for l, (w, b) in enumerate(zip(ws, bs)):
            m[f"w{l}"] = w
            m[f"b{l}"] = b
        in_maps.append(m)
    return in_maps


def _run(nc, in_maps, trace=False):
    from concourse.bass_utils import run_bass_kernel_spmd

    global _LAST_RESULTS
    try:
        res = run_bass_kernel_spmd(
            nc, in_maps, core_ids=list(range(N_CORES)), trace=trace
        )
    except ModuleNotFoundError:
        # NTFF profile hook unavailable in this container; rerun untraced.
        res = run_bass_kernel_spmd(
            nc, in_maps, core_ids=list(range(N_CORES)), trace=False
        )
    _LAST_RESULTS = res
    return res.results


def _layer_dims():
    return (
        [(FULL_D_IN, FULL_D_HID)]
        + [(FULL_D_HID, FULL_D_HID)] * FULL_N_HIDDEN_LAYERS
        + [(FULL_D_HID, FULL_N_CLASSES)]
    )


def _get_nc():
    N = FULL_N
    R = N // N_CORES
    layer_dims = _layer_dims()
    key = (N, R, tuple(layer_dims))
    if key not in _CACHE:
        _CACHE[key] = _build(N, R, layer_dims)
    return _CACHE[key]


def kernel(x, adj, W_in, b_in, W_hidden, b_hidden, W_out, b_out):
    N = FULL_N
    R = N // N_CORES
    nc = _get_nc()
    in_maps = _prep_inputs(
        x, adj, W_in, b_in, W_hidden, b_hidden, W_out, b_out, N, R
    )
    trace = os.environ.get("GCN_TRACE", "0") == "1"
    results = _run(nc, in_maps, trace=trace)
    out = np.empty((N, FULL_N_CLASSES), dtype=np.float32)
    for c in range(N_CORES):
        out[c * R : (c + 1) * R, :] = results[c]["outT"].T
    return out


# revision 25
# speedup vs baseline: 2.1405x; 2.1405x over previous
"""4-layer GCN (out = adj @ (h @ W) + b, stacked) on 8 trn2 NeuronCores.

Strategy (row-parallel over nodes, host-prepped adjacency):
  - Each core owns R = N/8 rows of adj (its output rows for every layer).
  - The adjacency is transposed and quantized ON THE HOST into a single
    fp8 e4m3 copy (scaled by 2^16 so values land in [0, 4]) streamed by
    ALL FOUR layers.  Measured on the harness data, fp8 adjacency
    everywhere costs ~7e-4 extra rel err (4.1e-3 -> 4.8e-3) -- the gate
    is 2e-2 -- while halving HBM traffic vs bf16.
  - Per layer the core streams its adjT shard in 2 MiB chunks and runs
    the big GEMM h^T = Z^T-contracted against adjT.  Most layers feed
    the PE MIXED operands: bf16 Z (stationary) x fp8 adjT (moving) --
    quantizing Z itself to e4m3 is measured to cost 2-3e-2 rel err on
    layers 0/2/3, so Z stays bf16 there.  Layer 1 tolerates fp8 Z
    (5e-3 measured), so it runs DoubleRow fp8 x fp8 (2 k-blocks per
    matmul, ~1.8x PE) with a 2^10 scale folded into its weights.
  - Z = h @ W is computed redundantly per core (tiny); the fp8 scales
    are removed by a mult fused into the PSUM->SBUF bias add
    (tensor_scalar mult+add).
  - h^T shards are AllGather'd (bf16) between layers.

kernel(**inputs) takes the full-size numpy inputs and returns the full
[N, 16] float32 output.
"""

import os

import numpy as np
import ml_dtypes

P = 128            # SBUF partitions / PE tile size
N_CORES = 8
SEG = 512          # fp32 PSUM bank width (free-dim elements)

# Full-problem config (must match the harness problem)
FULL_N = 16384
FULL_D_IN = 128
FULL_D_HID = 64
FULL_N_CLASSES = 16
FULL_N_HIDDEN_LAYERS = 2

ADJ_SHIFT = 16     # adjT8 = e4m3(adjT * 2^ADJ_SHIFT); adj max = 1/N = 2^-14
DR_LAYERS = (1,)   # layers running DoubleRow fp8 x fp8 (fp8-quantized Z)
SIGMA = {1: 10}    # Z' = Z * 2^sigma for DR layers (folded into W, bf16-exact)
CHUNK8 = 8         # fp8 k-blocks per strip DMA  (128p x 8 x 2048 x 1B = 2 MiB)

_CACHE = {}
_LAST_RESULTS = None  # BassKernelResults of the most recent run (for test.py)


def _split_dma_waits(nc, mybir, max_waits=1, noop_waits=1):
    """Walrus' DMA pseudo-instruction supports at most 2 sem waits; Tile can
    emit 3+.  Hoist all waits of offending DMAs onto a NoOp on the issuing
    engine immediately before the DMA (same NX stream, so ordering holds)."""
    for f in nc.m.functions:
        for bb in f.blocks:
            insts = bb.instructions
            i = 0
            while i < len(insts):
                ins = insts[i]
                si = ins.sync_info
                if (
                    si is not None
                    and si.on_wait
                    and len(si.on_wait) > max_waits
                ):
                    waits = list(si.on_wait)
                    keep = waits[-max_waits:]
                    extra = waits[:-max_waits]
                    for j in range(0, len(extra), noop_waits):
                        noop = mybir.InstNoOp(
                            name=nc.get_next_instruction_name(),
                            engine=ins.engine,
                            ins=[],
                            outs=[],
                            sync_info=mybir.SyncInfo(
                                on_wait=extra[j : j + noop_waits], on_update=[]
                            ),
                        )
                        insts.insert(i, noop)
                        i += 1
                    ins.sync_info = mybir.SyncInfo(
                        on_wait=keep, on_update=list(si.on_update or [])
                    )
                i += 1


def _build(N, R, layer_dims, collectives=True, split_waits=True):
    """Build the per-core Bass program.

    N: total nodes; R: rows per core; layer_dims: [(d_in, d_out), ...]
    """
    import concourse.bass as bass
    import concourse.mybir as mybir
    from concourse import tile

    f32 = mybir.dt.float32
    bf16 = mybir.dt.bfloat16
    fp8 = mybir.dt.float8e4

    KB = N // P                    # contraction k-blocks
    n_seg = R // SEG
    n_layers = len(layer_dims)
    d_in0 = layer_dims[0][0]
    d_last = layer_dims[-1][1]

    nc = bass.Bass(trn_type="TRN2", num_devices=N_CORES)

    adjT8_d = nc.dram_tensor("adjT8", [N, R], fp8, kind="ExternalInput")
    xT_d = nc.dram_tensor("xT", [d_in0, N], bf16, kind="ExternalInput")
    w_d = [
        nc.dram_tensor(f"w{l}", [di, do], bf16, kind="ExternalInput")
        for l, (di, do) in enumerate(layer_dims)
    ]
    b_d = [
        nc.dram_tensor(f"b{l}", [do, 1], f32, kind="ExternalInput")
        for l, (di, do) in enumerate(layer_dims)
    ]
    outT_d = nc.dram_tensor("outT", [d_last, R], f32, kind="ExternalOutput")

    with tile.TileContext(nc) as tc:
        with (
            tc.tile_pool(name="const", bufs=1) as constp,
            tc.tile_pool(name="xt", bufs=1) as xtp,
            tc.tile_pool(name="z16", bufs=1) as z16p,
            tc.tile_pool(name="s8", bufs=5) as s8p,
            tc.tile_pool(name="h", bufs=2) as hp,
            tc.tile_pool(name="hfull", bufs=1) as hfp,
            tc.tile_pool(name="pz", bufs=2, space="PSUM") as pzp,
            tc.tile_pool(name="ph", bufs=1, space="PSUM") as php,
            tc.tile_pool(name="dram", bufs=1, space="DRAM") as dramp,
        ):
            # First DMAs: xt slice 0 + w0 (the layer-0 Z stage deps), then
            # the packed w/b loads and remaining xt slices.
            xt = xtp.tile([d_in0, N], bf16, tag="xt")
            XSL = N // 8
            nc.sync.dma_start(xt[:, 0:XSL], xT_d[:, 0:XSL])
            w_sb, b_sb = [], []
            for l, (di, do) in enumerate(layer_dims):
                w = constp.tile([di, do], bf16, tag=f"w{l}")
                w_sb.append(w)
            for l in range(n_layers):
                nc.sync.dma_start(w_sb[l][:], w_d[l][:])
            for l, (di, do) in enumerate(layer_dims):
                b = constp.tile([do, 1], f32, tag=f"b{l}")
                nc.sync.dma_start(b[:], b_d[l][:])
                b_sb.append(b)
            for sl in range(1, 8):
                nc.sync.dma_start(
                    xt[:, sl * XSL : (sl + 1) * XSL],
                    xT_d[:, sl * XSL : (sl + 1) * XSL],
                )

            hT_bf = None  # gathered h^T [d, N] bf16 for layers >= 1
            for l in range(n_layers):
                di, do = layer_dims[l]
                last = l == n_layers - 1
                dr = l in DR_LAYERS

                # ---- Z_l = h_l @ W_l, [k-part, kb, do] layout ----
                # ZB k-blocks share one PSUM tile and one PSUM->SBUF copy,
                # so the stage is paced by matmuls, not per-block copies.
                hsrc = xt if l == 0 else hT_bf
                zbuf = z16p.tile([P, KB, do], fp8 if dr else bf16, tag="zbuf")
                ZB = 8
                for kb0 in range(0, KB, ZB):
                    pz = pzp.tile([P, ZB, do], f32, tag="pz")
                    for zi in range(ZB):
                        kb = kb0 + zi
                        nc.tensor.matmul(
                            pz[:, zi, :],
                            hsrc[:, kb * P : (kb + 1) * P],
                            w_sb[l][:],
                            start=True,
                            stop=True,
                        )
                    nc.any.tensor_copy(
                        zbuf[:, kb0 : kb0 + ZB, :], pz[:]
                    )

                # ---- big GEMM: h_{l+1}^T[n, i] = sum_k Z[k, n] adjT[k, i] ----
                ph = php.tile([do, R], f32, tag="ph")
                n_chunks = KB // CHUNK8
                for c in range(n_chunks):
                    kb0 = c * CHUNK8
                    strip = s8p.tile([P, CHUNK8, R], fp8, tag="s8")
                    nc.sync.dma_start(
                        strip[:],
                        adjT8_d[kb0 * P : (kb0 + CHUNK8) * P, :].rearrange(
                            "(kk p) r -> p kk r", p=P
                        ),
                    )
                    if dr:
                        for j in range(CHUNK8 // 2):
                            kb = kb0 + 2 * j
                            for s in range(n_seg):
                                nc.tensor.matmul(
                                    ph[:, s * SEG : (s + 1) * SEG],
                                    zbuf[:, kb : kb + 2, :],
                                    strip[:, 2 * j : 2 * j + 2,
                                          s * SEG : (s + 1) * SEG],
                                    perf_mode=mybir.MatmulPerfMode.DoubleRow,
                                    start=(kb == 0),
                                    stop=(kb == KB - 2),
                                )
                    else:
                        for j in range(CHUNK8):
                            kb = kb0 + j
                            for s in range(n_seg):
                                nc.tensor.matmul(
                                    ph[:, s * SEG : (s + 1) * SEG],
                                    zbuf[:, kb, :],
                                    strip[:, j, s * SEG : (s + 1) * SEG],
                                    start=(kb == 0),
                                    stop=(kb == KB - 1),
                                )

                # ---- descale + bias add and inter-layer AllGather ----
                descale = 2.0 ** -(ADJ_SHIFT + SIGMA.get(l, 0))
                if last:
                    hf = hp.tile([do, R], f32, tag="hf")
                    for s in range(n_seg):
                        nc.any.tensor_scalar(
                            hf[:, s * SEG : (s + 1) * SEG],
                            ph[:, s * SEG : (s + 1) * SEG],
                            descale,
                            b_sb[l][:, 0:1],
                            op0=mybir.AluOpType.mult,
                            op1=mybir.AluOpType.add,
                        )
                    nc.sync.dma_start(outT_d[:], hf[:])
                else:
                    hb = hp.tile([do, R], bf16, tag="hb")
                    for s in range(n_seg):
                        nc.any.tensor_scalar(
                            hb[:, s * SEG : (s + 1) * SEG],
                            ph[:, s * SEG : (s + 1) * SEG],
                            descale,
                            b_sb[l][:, 0:1],
                            op0=mybir.AluOpType.mult,
                            op1=mybir.AluOpType.add,
                        )
                    if collectives:
                        cc_in = dramp.tile([do, R], bf16, tag=f"ccin{l}")
                        nc.sync.dma_start(cc_in[:], hb[:])
                        cc_out = dramp.tile(
                            [N_CORES * do, R], bf16, addr_space="Shared",
                            tag=f"ccout{l}",
                        )
                        nc.gpsimd.collective_compute(
                            "AllGather",
                            mybir.AluOpType.bypass,
                            replica_groups=[list(range(N_CORES))],
                            ins=[cc_in.opt()],
                            outs=[cc_out.opt()],
                        )
                        hT_bf = hfp.tile([do, N], bf16, tag="hfull")
                        for r in range(N_CORES):
                            nc.sync.dma_start(
                                hT_bf[:, r * R : (r + 1) * R],
                                cc_out[r * do : (r + 1) * do, :],
                            )
                    else:
                        # single-core timing build: fake the gather with a
                        # DRAM round-trip of the same shape
                        cc_in = dramp.tile([do, R], bf16, tag=f"ccin{l}")
                        nc.sync.dma_start(cc_in[:], hb[:])
                        hT_bf = hfp.tile([do, N], bf16, tag="hfull")
                        for r in range(N_CORES):
                            nc.sync.dma_start(
                                hT_bf[:, r * R : (r + 1) * R], cc_in[:]
                            )
    if split_waits:
        _split_dma_waits(nc, mybir)
    return nc


def _prep_inputs(x, adj, W_in, b_in, W_hidden, b_hidden, W_out, b_out, N, R):
    bf = ml_dtypes.bfloat16
    f8 = ml_dtypes.float8_e4m3
    xT = np.ascontiguousarray(np.asarray(x, dtype=np.float32).T).astype(bf)
    ws = (
        [np.asarray(W_in)]
        + [np.asarray(W_hidden)[i] for i in range(np.asarray(W_hidden).shape[0])]
        + [np.asarray(W_out)]
    )
    bs = (
        [np.asarray(b_in)]
        + [np.asarray(b_hidden)[i] for i in range(np.asarray(b_hidden).shape[0])]
        + [np.asarray(b_out)]
    )
    # fold the DR layers' Z fp8 scale into the (power-of-2 exact) bf16 weights
    ws = [
        np.ascontiguousarray(
            w.astype(np.float32) * (2.0 ** SIGMA.get(l, 0))
        ).astype(bf)
        for l, w in enumerate(ws)
    ]
    bs = [np.ascontiguousarray(b.astype(np.float32).reshape(-1, 1)) for b in bs]

    adjT = np.asarray(adj, dtype=np.float32).T
    adjT8 = (adjT * float(2.0 ** ADJ_SHIFT)).astype(f8)
    in_maps = []
    for c in range(N_CORES):
        m = {
            "adjT8": np.ascontiguousarray(adjT8[:, c * R : (c + 1) * R]),
            "xT": xT,
        }
        for l, (w, b) in enumerate(zip(ws, bs)):
            m[f"w{l}"] = w
            m[f"b{l}"] = b
        in_maps.append(m)
    return in_maps


def _run(nc, in_maps, trace=False):
    from concourse.bass_utils import run_bass_kernel_spmd

    global _LAST_RESULTS
    try:
        res = run_bass_kernel_spmd(
            nc, in_maps, core_ids=list(range(N_CORES)), trace=trace
        )
    except ModuleNotFoundError:
        # NTFF profile hook unavailable in this container; rerun untraced.
        res = run_bass_kernel_spmd(
            nc, in_maps, core_ids=list(range(N_CORES)), trace=False
        )
    _LAST_RESULTS = res
    return res.results


def _layer_dims():
    return (
        [(FULL_D_IN, FULL_D_HID)]
        + [(FULL_D_HID, FULL_D_HID)] * FULL_N_HIDDEN_LAYERS
        + [(FULL_D_HID, FULL_N_CLASSES)]
    )


def _get_nc():
    N = FULL_N
    R = N // N_CORES
    layer_dims = _layer_dims()
    key = (N, R, tuple(layer_dims))
    if key not in _CACHE:
        _CACHE[key] = _build(N, R, layer_dims)
    return _CACHE[key]


def kernel(x, adj, W_in, b_in, W_hidden, b_hidden, W_out, b_out):
    N = FULL_N
    R = N // N_CORES
    nc = _get_nc()
    in_maps = _prep_inputs(
        x, adj, W_in, b_in, W_hidden, b_hidden, W_out, b_out, N, R
    )
    trace = os.environ.get("GCN_TRACE", "0") == "1"
    results = _run(nc, in_maps, trace=trace)
    out = np.empty((N, FULL_N_CLASSES), dtype=np.float32)
    for c in range(N_CORES):
        out[c * R : (c + 1) * R, :] = results[c]["outT"].T
    return out
